# revision 52
# baseline (speedup 1.0000x reference)
"""Trainium2 Bass kernel for the nn_Attention problem (non-local attention block).

Reference computation (per batch b, with N = W*H spatial positions):
    q = wq @ r + bq                # [Co, N] from range_x
    k = wk @ i + bk                # [Co, N] from img
    corr[n, m] = q[:, n] . k[:, m]
    attn = softmax_m(corr)
    v = wv @ i + bv
    out = v @ attn^T               # [Co, N]
    y = relu(BN(wc @ out + bc))
    result = img + y

Algebraic restructuring used here:
    corr[n, m] = r_n^T A i_m + u[m] + w[n] + const,  A = wq^T wk
  - w[n] and const are per-query-row constants -> cancel in softmax. Dropped.
  - u[m] = (wk^T bq) . i_m varies per key -> kept, folded into Vhat as a
    per-key e^u scaling (exp(l+u) = exp(l) * e^u).
  - P = A @ i is precomputed on-device ([C, N]); logits tile = P_tile^T @ r
    contracts over C=128, fully using the 128-deep PE array (the naive
    q^T k contraction is only Co=64 deep).
  - softmax max-subtraction is skipped: logits ~ N(0, 64), max |logit| < ~60,
    exp stays comfortably inside fp32/bf16 range.
  - Normalization is deferred: Vhat = [v * e^u ; e^u] (65 rows). Then
    out_un = Vhat^T @ E where E = exp(P^T r); row 64 of out_un is the
    softmax denominator. v's bias bv is folded into the final projection
    bias on host.
  - BN (inference) + conv biases fold into wc' and bc'' on host.

Precision (validated numerically, end-to-end rel err ~5e-3):
  - logits path in fp16 (fp32 matmuls lower to TWO 2-cycle/column passes on
    the PE -> 4x the cost of a 16-bit matmul; fp16's 10-bit mantissa keeps
    the logit error ~8x below bf16)
  - attention-value path in bf16 (E spans e^-60..e^60: needs bf16 range)
  - softmax denominator/normalization in fp32

Sharding: 8 cores = 4 batches x 2 query-halves. Keys/values are the full
N=4096 per core; queries are a 2048-slice. No cross-core communication.

Engine discipline: walrus allows only ONE semaphore sync-wait per ISA
instruction (Bacc legalizes overflow into EventSemaphore preludes, but each
EVSEM costs an issue slot on the engine) so the kernel is structured so
nearly every instruction has at most one fresh semaphore dependency:
preamble PSUM->SBUF copies run on ACT, the postamble runs on DVE, every
PSUM tile gets one full-range reader, et tiles are never recycled within a
chunk, and a per-chunk ACT "absorber" advances ACT's observed self-tick so
cross-chunk et-slot WAW waits elide.
"""

import numpy as np

BN_EPS = 1e-5

_CACHE: dict = {}
_LAST_RESULTS = None  # BassKernelResults of the most recent run (for profiling)

# packed fp32 input layout (elements per partition row)
_OFF32_IMQ = 0       # [128, 2048] residual img (query half)
_OFF32_BCC = 2048    # [128, 1] folded output bias
_XIN32_W = 2049
# packed fp16 input layout
_OFF16_IMG = 0       # [128, 4096]
_OFF16_RNG = 4096    # [128, 2048]
_OFF16_AT = 6144     # [128, 128]
_OFF16_WVG = 6272    # [128, 65]
_XIN16_W = 6337
# packed bf16 input layout
_OFFBF_WCT = 0       # [64, 128]
_XINBF_W = 128


def _build_program(C: int, N: int, NQ: int, Co: int):
    import concourse.bass as bass
    import concourse.tile as tile
    from concourse import bacc, mybir

    f32 = mybir.dt.float32
    f16 = mybir.dt.float16
    bf16 = mybir.dt.bfloat16
    Exp = mybir.ActivationFunctionType.Exp
    Copy = mybir.ActivationFunctionType.Copy
    Add = mybir.AluOpType.add
    Max = mybir.AluOpType.max

    MT = N // 128      # key tiles (32)
    NCH = NQ // 512    # query chunks per core (4)
    PCH = N // 512     # chunks for P generation (8)

    # Bacc (not raw Bass): its compile() legalizes multi-wait instructions
    # into EventSemaphore preludes (TRN2 allows 1 sync wait per instruction).
    nc = bacc.Bacc()
    x32_d = nc.declare_dram_parameter("x32", [C, _XIN32_W], f32, isOutput=False)
    x16_d = nc.declare_dram_parameter("x16", [C, _XIN16_W], f16, isOutput=False)
    xbf_d = nc.declare_dram_parameter("xbf", [C, _XINBF_W], bf16, isOutput=False)
    out_d = nc.declare_dram_parameter("out", [C, NQ], f32, isOutput=True)

    with tile.TileContext(nc) as tc:
        with (
            tc.tile_pool(name="const", bufs=1) as cpool,
            tc.tile_pool(name="work", bufs=3) as wpool,
            # One et slot per exp within a chunk: an ACT->ACT WAW on a
            # recycled et slot would emit an ACT self-wait on the exp.
            tc.tile_pool(name="et", bufs=MT // 2) as epool,
            # PSUM: "pc" 2x2 banks (1024-wide logit tiles halve the ACT
            # per-instruction overhead), "pvy" 1, "po" 3 (postambles run two
            # chunks behind the main loop) -> 8 of 8.
            tc.tile_pool(name="ps_c", bufs=2, space="PSUM") as ps_c,
            tc.tile_pool(name="ps_vy", bufs=1, space="PSUM") as ps_vy,
            tc.tile_pool(name="ps_o", bufs=3, space="PSUM") as ps_o,
        ):
            # Chunked input DMAs: weights+queries first (small), then img in
            # quarters, so stage A starts ~2us in instead of after one big
            # 16us transfer. Separate DMA instructions land on separate HWDGE
            # queues and run in parallel.
            x16 = cpool.tile([C, _XIN16_W], f16)
            nc.sync.dma_start(x16[:, _OFF16_RNG:], x16_d[:, _OFF16_RNG:])
            for j in range(4):
                nc.sync.dma_start(
                    x16[:, j * 1024:(j + 1) * 1024],
                    x16_d[:, j * 1024:(j + 1) * 1024],
                )
            x32 = cpool.tile([C, _XIN32_W], f32)
            nc.sync.dma_start(x32[:], x32_d[:])
            xbf = cpool.tile([C, _XINBF_W], bf16)
            nc.sync.dma_start(xbf[:], xbf_d[:])
            imq_t = x32[:, _OFF32_IMQ:_OFF32_IMQ + NQ]
            bcc_t = x32[:, _OFF32_BCC:_OFF32_BCC + 1]
            img_t = x16[:, _OFF16_IMG:_OFF16_IMG + N]
            rng_t = x16[:, _OFF16_RNG:_OFF16_RNG + NQ]
            at_t = x16[:, _OFF16_AT:_OFF16_AT + C]
            wvg_t = x16[:, _OFF16_WVG:_OFF16_WVG + Co + 1]
            wct_t = xbf[0:Co, _OFFBF_WCT:_OFFBF_WCT + C]

            p_t = cpool.tile([C, N], f16)          # P = A @ img
            vhat_t = cpool.tile([128, MT, Co + 1], bf16)  # [v*eu ; eu] key-major
            eu_t = cpool.tile([128, MT], f32)      # e^u (fp32, for ACT scale)
            ab_src = cpool.tile([1, 1], f32)       # absorber scratch (see below)
            ab_dst = cpool.tile([1, NCH], f32)
            # single staging buffer + one output DMA (fewer HWDGE queue sems)
            fin_all = cpool.tile([C, NQ], f32)
            ones_t = cpool.tile([1, Co], f32)
            nc.vector.memset(ones_t[:], 1.0)
            bcc2_t = cpool.tile([C, 1], f32)
            nc.vector.tensor_copy(bcc2_t[:], bcc_t)
            nc.scalar.activation(ab_src[:], x32[0:1, 0:1], Copy, scale=0.0)

            # PE pre-warm: dummy matmuls (input = a DVE-memset row, no DMA
            # dependency) run during the instruction-upload/DMA window so the
            # HAM clock gate is at 2.4GHz when real work starts.
            warm_t = cpool.tile([1, 512], f32)
            nc.vector.memset(warm_t[:], 0.0)
            for _ in range(14):
                pw = ps_vy.tile([Co, 512], f32, tag="pvy")
                nc.tensor.matmul(pw, ones_t[:], warm_t[:], start=True, stop=True)

            # Stage A: P = A @ img  (lhsT = A^T, stationary across all chunks)
            # NOTE: each PSUM tile needs one reader covering the FULL written
            # range, else Tile keeps the PE writer in the slot release set and
            # the next matmul on that slot carries two sem waits.
            for j in range(PCH // 2):
                ps = ps_c.tile([128, 1024], f32, tag="pc")
                for k in range(2):
                    nc.tensor.matmul(
                        ps[:, k * 512:(k + 1) * 512], at_t,
                        img_t[:, (2 * j + k) * 512:(2 * j + k + 1) * 512],
                        start=True, stop=True,
                    )
                nc.vector.tensor_copy(p_t[:, j * 1024:(j + 1) * 1024], ps[:])

            # Stage B: per key-tile t: [v^T | u] = img_tile^T @ [wv^T | g],
            # then vhat = [v * e^u | e^u]. Seven tile outputs are packed per
            # PSUM bank (7 x 65 x 4B < 2KB) so the PSUM slot round-trip count
            # is 5 instead of 16 -- with per-tile round trips this stage
            # dribbled through the main loop and stalled the po matmuls.
            t0 = 0
            while t0 < MT:
                cnt = min(7, MT - t0)
                pvb = ps_vy.tile([128, 512], f32, tag="pvy")
                for i in range(cnt):
                    nc.tensor.matmul(
                        pvb[:, i * 65:(i + 1) * 65],
                        img_t[:, (t0 + i) * 128:(t0 + i + 1) * 128], wvg_t,
                        start=True, stop=True,
                    )
                pv3 = pvb[:, 0:cnt * 65].rearrange("p (t c) -> p t c", c=65)
                nc.vector.tensor_copy(
                    vhat_t[:, t0:t0 + cnt, :].rearrange("p t c -> p (t c)"),
                    pvb[:, 0:cnt * 65],
                )
                nc.scalar.activation(eu_t[:, t0:t0 + cnt], pv3[:, :, Co], Exp)
                nc.vector.tensor_copy(
                    vhat_t[:, t0:t0 + cnt, Co], eu_t[:, t0:t0 + cnt]
                )
                for i in range(cnt):
                    t = t0 + i
                    nc.vector.tensor_scalar_mul(
                        vhat_t[:, t, 0:Co], vhat_t[:, t, 0:Co], eu_t[:, t:t + 1]
                    )
                t0 += cnt

            def postamble(ch, po, pe_pin):
                # Normalize, project, BN+relu, residual (DVE + PE). Runs
                # software-pipelined two chunks behind the main loop; the two
                # PE matmuls are additionally pinned (no-sync ordering edges)
                # after `pe_pin` so they never stall PE's stream behind the
                # DVE reciprocal chain (a >3.4us PE gap re-throttles the HAM
                # clock gate).
                recip = wpool.tile([1, 512], f32, tag="recip")
                nc.vector.reciprocal(recip[:], po[Co:Co + 1, :])
                # Broadcast recip across partitions via a K=1 PE outer product.
                po2 = ps_vy.tile([Co, 512], f32, tag="pvy")
                mm1 = nc.tensor.matmul(po2, ones_t[:], recip[:],
                                       start=True, stop=True)
                rb = wpool.tile([Co, 512], f32, tag="rb")
                nc.vector.tensor_copy(rb[:], po2[:])
                onorm = wpool.tile([Co, 512], bf16, tag="onorm")
                nc.vector.tensor_mul(onorm[:], po[0:Co, :], rb[:])
                py = ps_vy.tile([128, 512], f32, tag="pvy")
                mm2 = nc.tensor.matmul(py, wct_t, onorm[:], start=True, stop=True)
                if pe_pin is not None:
                    for mm in (mm1, mm2):
                        tile.add_dep_helper(
                            mm.ins, pe_pin.ins, sync=False,
                            reason="postamble PE after next-next chunk start",
                        )
                yr = wpool.tile([128, 512], f32, tag="yr")
                nc.vector.tensor_scalar(yr[:], py[:], bcc2_t[:], 0.0, Add, Max)
                nc.vector.tensor_add(
                    fin_all[:, ch * 512:(ch + 1) * 512], yr[:],
                    imq_t[:, ch * 512:(ch + 1) * 512],
                )
                nc.sync.dma_start(
                    out_d[:, ch * 512:(ch + 1) * 512],
                    fin_all[:, ch * 512:(ch + 1) * 512],
                )

            # Main loop: per query chunk, accumulate Vhat^T @ exp(P^T r)
            prev_absorber = None
            pending = []
            for ch in range(NCH):
                po = ps_o.tile([Co + 1, 512], f32, tag="po")
                last_exp = None
                first_corr = None
                for tt in range(MT // 2):
                    pc = ps_c.tile([128, 1024], f32, tag="pc")
                    for k in range(2):
                        mm = nc.tensor.matmul(
                            pc[:, k * 512:(k + 1) * 512],
                            p_t[:, (2 * tt + k) * 128:(2 * tt + k + 1) * 128],
                            rng_t[:, ch * 512:(ch + 1) * 512],
                            start=True, stop=True,
                        )
                        if first_corr is None:
                            first_corr = mm
                    et = epool.tile([128, 1024], bf16)
                    second_last_exp = last_exp
                    last_exp = nc.scalar.activation(et, pc, Exp)
                    if prev_absorber is not None:
                        # order this chunk's exps after the previous chunk's
                        # absorber (same engine; no semaphore emitted)
                        tile.add_dep_helper(
                            last_exp.ins, prev_absorber.ins, sync=False,
                            reason="exp after absorber",
                        )
                    for k in range(2):
                        t = 2 * tt + k
                        nc.tensor.matmul(
                            po, vhat_t[:, t, :], et[:, k * 512:(k + 1) * 512],
                            start=(t == 0), stop=(t == MT - 1),
                        )
                # ACT self-tick absorber: a sync edge to the chunk's second-to-
                # last exp advances ACT's observed self-semaphore past (almost)
                # every exp of this chunk, so next-chunk et-slot WAW waits are
                # elided (the one uncovered exp costs a single EVSEM). Waiting
                # on the second-to-last exp hides the completion-semaphore
                # round-trip behind the last exp's execution.
                absorber = nc.scalar.copy(ab_dst[0:1, ch:ch + 1], ab_src[:])
                tile.add_dep_helper(
                    absorber.ins, second_last_exp.ins, sync=True,
                    reason="ACT self-tick absorber",
                )
                prev_absorber = absorber
                pending.append((ch, po))
                if len(pending) > 2:
                    postamble(*pending.pop(0), pe_pin=first_corr)
                if ch == NCH - 1:
                    # overlap all but the final postamble with this chunk
                    while len(pending) > 1:
                        postamble(*pending.pop(0), pe_pin=first_corr)
            postamble(*pending.pop(0), pe_pin=None)

    # Bacc defers register allocation etc. to compile(); finalize() runs it.
    nc.finalize()
    return nc


def _prepare(range_x, img, wq, bq, wk, bk, wv, bv, wc, bc,
             bn_gamma, bn_beta, bn_mean, bn_var):
    """Build (or fetch) the Bass program and the 8 per-core input maps."""
    import sys
    if "/opt/trn_rl_repo" not in sys.path:
        sys.path.insert(0, "/opt/trn_rl_repo")
    import ml_dtypes

    range_x = np.asarray(range_x, np.float32)
    img = np.asarray(img, np.float32)
    wq = np.asarray(wq, np.float32)
    bq = np.asarray(bq, np.float32)
    wk = np.asarray(wk, np.float32)
    bk = np.asarray(bk, np.float32)
    wv = np.asarray(wv, np.float32)
    bv = np.asarray(bv, np.float32)
    wc = np.asarray(wc, np.float32)
    bc = np.asarray(bc, np.float32)
    bn_gamma = np.asarray(bn_gamma, np.float32)
    bn_beta = np.asarray(bn_beta, np.float32)
    bn_mean = np.asarray(bn_mean, np.float32)
    bn_var = np.asarray(bn_var, np.float32)

    B, C, W, H = range_x.shape
    N = W * H
    NQ = N // 2
    Co = wq.shape[0]

    # Host-side weight folding (all tiny).
    inv = bn_gamma / np.sqrt(bn_var + BN_EPS)
    wcp = inv[:, None] * wc                                   # [C, Co]
    bcc = inv * bc + bn_beta - bn_mean * inv + wcp @ bv       # [C]
    at = wk.T @ wq                                            # lhsT for P-gen
    wvg = np.concatenate([wv.T, (wk.T @ bq)[:, None]], axis=1)  # [C, Co+1]
    wct = wcp.T                                               # [Co, C]

    key = (C, N, NQ, Co)
    if key not in _CACHE:
        _CACHE[key] = _build_program(C, N, NQ, Co)
    nc = _CACHE[key]

    n_cores = 8
    in_maps = []
    for core in range(n_cores):
        b, h = core // 2, core % 2
        im = img[b].reshape(C, N)
        x32 = np.zeros((C, _XIN32_W), np.float32)
        x32[:, _OFF32_IMQ:_OFF32_IMQ + NQ] = im[:, h * NQ:(h + 1) * NQ]
        x32[:, _OFF32_BCC] = bcc
        x16 = np.zeros((C, _XIN16_W), np.float16)
        x16[:, _OFF16_IMG:_OFF16_IMG + N] = im
        x16[:, _OFF16_RNG:_OFF16_RNG + NQ] = \
            range_x[b].reshape(C, N)[:, h * NQ:(h + 1) * NQ]
        x16[:, _OFF16_AT:_OFF16_AT + C] = at
        x16[:, _OFF16_WVG:_OFF16_WVG + Co + 1] = wvg
        xbf = np.zeros((C, _XINBF_W), ml_dtypes.bfloat16)
        xbf[0:Co, _OFFBF_WCT:_OFFBF_WCT + C] = wct.astype(ml_dtypes.bfloat16)
        in_maps.append({"x32": x32, "x16": x16, "xbf": xbf})

    return nc, in_maps, (B, C, W, H, N, NQ)


def kernel(range_x, img, wq, bq, wk, bk, wv, bv, wc, bc,
           bn_gamma, bn_beta, bn_mean, bn_var):
    import sys
    if "/opt/trn_rl_repo" not in sys.path:
        sys.path.insert(0, "/opt/trn_rl_repo")
    from concourse.bass_utils import run_bass_kernel_spmd

    nc, in_maps, (B, C, W, H, N, NQ) = _prepare(
        range_x, img, wq, bq, wk, bk, wv, bv, wc, bc,
        bn_gamma, bn_beta, bn_mean, bn_var)

    global _LAST_RESULTS
    _LAST_RESULTS = run_bass_kernel_spmd(nc, in_maps, list(range(8)))
    res = _LAST_RESULTS.results

    out = np.empty((B, C, N), np.float32)
    for core in range(8):
        b, h = core // 2, core % 2
        out[b, :, h * NQ:(h + 1) * NQ] = res[core]["out"]
    return out.reshape(B, C, W, H)


# revision 53
# speedup vs baseline: 1.1878x; 1.1878x over previous
"""Trainium2 Bass kernel for the nn_Attention problem (non-local attention block).

Reference computation (per batch b, with N = W*H spatial positions):
    q = wq @ r + bq                # [Co, N] from range_x
    k = wk @ i + bk                # [Co, N] from img
    corr[n, m] = q[:, n] . k[:, m]
    attn = softmax_m(corr)
    v = wv @ i + bv
    out = v @ attn^T               # [Co, N]
    y = relu(BN(wc @ out + bc))
    result = img + y

Algebraic restructuring used here:
    corr[n, m] = r_n^T A i_m + u[m] + w[n] + const,  A = wq^T wk
  - w[n] and const are per-query-row constants -> cancel in softmax. Dropped.
  - u[m] = (wk^T bq) . i_m varies per key -> kept, folded into Vhat as a
    per-key e^u scaling (exp(l+u) = exp(l) * e^u).
  - P = A @ i is precomputed on-device ([C, N]); logits tile = P_tile^T @ r
    contracts over C=128, fully using the 128-deep PE array (the naive
    q^T k contraction is only Co=64 deep).
  - softmax max-subtraction is skipped: logits ~ N(0, 64), max |logit| < ~60,
    exp stays comfortably inside fp32/bf16 range.
  - Normalization is deferred: Vhat = [v * e^u ; e^u] (65 rows). Then
    out_un = Vhat^T @ E where E = exp(P^T r); row 64 of out_un is the
    softmax denominator. v's bias bv is folded into the final projection
    bias on host.
  - BN (inference) + conv biases fold into wc' and bc'' on host.

Precision (validated numerically, end-to-end rel err ~5e-3):
  - logits path in fp16 (fp32 matmuls lower to TWO 2-cycle/column passes on
    the PE -> 4x the cost of a 16-bit matmul; fp16's 10-bit mantissa keeps
    the logit error ~8x below bf16)
  - attention-value path in bf16 (E spans e^-60..e^60: needs bf16 range)
  - softmax denominator/normalization in fp32

Sharding: 8 cores = 4 batches x 2 query-halves. Keys/values are the full
N=4096 per core; queries are a 2048-slice. No cross-core communication.

Engine discipline: walrus allows only ONE semaphore sync-wait per ISA
instruction (Bacc legalizes overflow into EventSemaphore preludes, but each
EVSEM costs an issue slot on the engine) so the kernel is structured so
nearly every instruction has at most one fresh semaphore dependency:
preamble PSUM->SBUF copies run on ACT, the postamble runs on DVE, every
PSUM tile gets one full-range reader, et tiles are never recycled within a
chunk, and a per-chunk ACT "absorber" advances ACT's observed self-tick so
cross-chunk et-slot WAW waits elide.
"""

import numpy as np

BN_EPS = 1e-5

_CACHE: dict = {}
_LAST_RESULTS = None  # BassKernelResults of the most recent run (for profiling)

# packed fp32 input layout (elements per partition row)
_OFF32_IMQ = 0       # [128, 2048] residual img (query half)
_OFF32_BCC = 2048    # [128, 1] folded output bias
_XIN32_W = 2049
# packed fp16 input layout
_OFF16_IMG = 0       # [128, 4096]
_OFF16_RNG = 4096    # [128, 2048]
_OFF16_AT = 6144     # [128, 128]
_OFF16_WVG = 6272    # [128, 65]
_XIN16_W = 6337
# packed bf16 input layout
_OFFBF_WCT = 0       # [64, 128]
_XINBF_W = 128


def _build_program(C: int, N: int, NQ: int, Co: int):
    import concourse.bass as bass
    import concourse.tile as tile
    from concourse import bacc, mybir

    f32 = mybir.dt.float32
    f16 = mybir.dt.float16
    bf16 = mybir.dt.bfloat16
    Exp = mybir.ActivationFunctionType.Exp
    Copy = mybir.ActivationFunctionType.Copy
    Add = mybir.AluOpType.add
    Max = mybir.AluOpType.max

    MT = N // 128      # key tiles (32)
    NCH = NQ // 512    # query chunks per core (4)
    PCH = N // 512     # chunks for P generation (8)

    # Bacc (not raw Bass): its compile() legalizes multi-wait instructions
    # into EventSemaphore preludes (TRN2 allows 1 sync wait per instruction).
    nc = bacc.Bacc()
    x32_d = nc.declare_dram_parameter("x32", [C, _XIN32_W], f32, isOutput=False)
    x16_d = nc.declare_dram_parameter("x16", [C, _XIN16_W], f16, isOutput=False)
    xbf_d = nc.declare_dram_parameter("xbf", [C, _XINBF_W], bf16, isOutput=False)
    out_d = nc.declare_dram_parameter("out", [C, NQ], f32, isOutput=True)

    with tile.TileContext(nc) as tc:
        with (
            tc.tile_pool(name="const", bufs=1) as cpool,
            tc.tile_pool(name="work", bufs=3) as wpool,
            # One et slot per exp within a chunk: an ACT->ACT WAW on a
            # recycled et slot would emit an ACT self-wait on the exp.
            tc.tile_pool(name="et", bufs=MT // 2) as epool,
            # PSUM: "pc" 2x2 banks (1024-wide logit tiles halve the ACT
            # per-instruction overhead), "pvy" 1, "po" 3 (postambles run two
            # chunks behind the main loop) -> 8 of 8.
            tc.tile_pool(name="ps_c", bufs=2, space="PSUM") as ps_c,
            tc.tile_pool(name="ps_vy", bufs=1, space="PSUM") as ps_vy,
            tc.tile_pool(name="ps_o", bufs=3, space="PSUM") as ps_o,
        ):
            # Chunked input DMAs: weights+queries first (small), then img in
            # quarters, so stage A starts ~2us in instead of after one big
            # 16us transfer. Separate DMA instructions land on separate HWDGE
            # queues and run in parallel.
            x16 = cpool.tile([C, _XIN16_W], f16)
            nc.sync.dma_start(x16[:, _OFF16_RNG:], x16_d[:, _OFF16_RNG:])
            for j in range(4):
                nc.sync.dma_start(
                    x16[:, j * 1024:(j + 1) * 1024],
                    x16_d[:, j * 1024:(j + 1) * 1024],
                )
            x32 = cpool.tile([C, _XIN32_W], f32)
            nc.sync.dma_start(x32[:], x32_d[:])
            xbf = cpool.tile([C, _XINBF_W], bf16)
            nc.sync.dma_start(xbf[:], xbf_d[:])
            imq_t = x32[:, _OFF32_IMQ:_OFF32_IMQ + NQ]
            bcc_t = x32[:, _OFF32_BCC:_OFF32_BCC + 1]
            img_t = x16[:, _OFF16_IMG:_OFF16_IMG + N]
            rng_t = x16[:, _OFF16_RNG:_OFF16_RNG + NQ]
            at_t = x16[:, _OFF16_AT:_OFF16_AT + C]
            wvg_t = x16[:, _OFF16_WVG:_OFF16_WVG + Co + 1]
            wct_t = xbf[0:Co, _OFFBF_WCT:_OFFBF_WCT + C]

            p_t = cpool.tile([C, N], f16)          # P = A @ img
            vhat_t = cpool.tile([128, MT, Co + 1], bf16)  # [v*eu ; eu] key-major
            eu_t = cpool.tile([128, MT], f32)      # e^u (fp32, for ACT scale)
            ab_src = cpool.tile([1, 1], f32)       # absorber scratch (see below)
            ab_dst = cpool.tile([1, NCH], f32)
            # single staging buffer + one output DMA (fewer HWDGE queue sems)
            fin_all = cpool.tile([C, NQ], f32)
            ones_t = cpool.tile([1, Co], f32)
            nc.vector.memset(ones_t[:], 1.0)
            bcc2_t = cpool.tile([C, 1], f32)
            nc.vector.tensor_copy(bcc2_t[:], bcc_t)
            nc.scalar.activation(ab_src[:], x32[0:1, 0:1], Copy, scale=0.0)



            # Stage A: P = A @ img  (lhsT = A^T, stationary across all chunks)
            # NOTE: each PSUM tile needs one reader covering the FULL written
            # range, else Tile keeps the PE writer in the slot release set and
            # the next matmul on that slot carries two sem waits.
            for j in range(PCH // 2):
                ps = ps_c.tile([128, 1024], f32, tag="pc")
                for k in range(2):
                    nc.tensor.matmul(
                        ps[:, k * 512:(k + 1) * 512], at_t,
                        img_t[:, (2 * j + k) * 512:(2 * j + k + 1) * 512],
                        start=True, stop=True,
                    )
                nc.vector.tensor_copy(p_t[:, j * 1024:(j + 1) * 1024], ps[:])

            # Stage B: per key-tile t: [v^T | u] = img_tile^T @ [wv^T | g],
            # then vhat = [v * e^u | e^u]. Seven tile outputs are packed per
            # PSUM bank (7 x 65 x 4B < 2KB) so the PSUM slot round-trip count
            # is 5 instead of 16 -- with per-tile round trips this stage
            # dribbled through the main loop and stalled the po matmuls.
            t0 = 0
            while t0 < MT:
                cnt = min(7, MT - t0)
                pvb = ps_vy.tile([128, 512], f32, tag="pvy")
                for i in range(cnt):
                    nc.tensor.matmul(
                        pvb[:, i * 65:(i + 1) * 65],
                        img_t[:, (t0 + i) * 128:(t0 + i + 1) * 128], wvg_t,
                        start=True, stop=True,
                    )
                pv3 = pvb[:, 0:cnt * 65].rearrange("p (t c) -> p t c", c=65)
                nc.vector.tensor_copy(
                    vhat_t[:, t0:t0 + cnt, :].rearrange("p t c -> p (t c)"),
                    pvb[:, 0:cnt * 65],
                )
                nc.scalar.activation(eu_t[:, t0:t0 + cnt], pv3[:, :, Co], Exp)
                nc.vector.tensor_copy(
                    vhat_t[:, t0:t0 + cnt, Co], eu_t[:, t0:t0 + cnt]
                )
                for i in range(cnt):
                    t = t0 + i
                    nc.vector.tensor_scalar_mul(
                        vhat_t[:, t, 0:Co], vhat_t[:, t, 0:Co], eu_t[:, t:t + 1]
                    )
                t0 += cnt

            def postamble(ch, po, pe_pin):
                # Normalize, project, BN+relu, residual (DVE + PE). Runs
                # software-pipelined two chunks behind the main loop; the two
                # PE matmuls are additionally pinned (no-sync ordering edges)
                # after `pe_pin` so they never stall PE's stream behind the
                # DVE reciprocal chain (a >3.4us PE gap re-throttles the HAM
                # clock gate).
                recip = wpool.tile([1, 512], f32, tag="recip")
                nc.vector.reciprocal(recip[:], po[Co:Co + 1, :])
                # Broadcast recip across partitions via a K=1 PE outer product.
                po2 = ps_vy.tile([Co, 512], f32, tag="pvy")
                mm1 = nc.tensor.matmul(po2, ones_t[:], recip[:],
                                       start=True, stop=True)
                rb = wpool.tile([Co, 512], f32, tag="rb")
                nc.vector.tensor_copy(rb[:], po2[:])
                onorm = wpool.tile([Co, 512], bf16, tag="onorm")
                nc.vector.tensor_mul(onorm[:], po[0:Co, :], rb[:])
                py = ps_vy.tile([128, 512], f32, tag="pvy")
                mm2 = nc.tensor.matmul(py, wct_t, onorm[:], start=True, stop=True)
                if pe_pin is not None:
                    for mm in (mm1, mm2):
                        tile.add_dep_helper(
                            mm.ins, pe_pin.ins, sync=False,
                            reason="postamble PE after next-next chunk start",
                        )
                yr = wpool.tile([128, 512], f32, tag="yr")
                nc.vector.tensor_scalar(yr[:], py[:], bcc2_t[:], 0.0, Add, Max)
                nc.vector.tensor_add(
                    fin_all[:, ch * 512:(ch + 1) * 512], yr[:],
                    imq_t[:, ch * 512:(ch + 1) * 512],
                )
                nc.sync.dma_start(
                    out_d[:, ch * 512:(ch + 1) * 512],
                    fin_all[:, ch * 512:(ch + 1) * 512],
                )

            # Main loop: per query chunk, accumulate Vhat^T @ exp(P^T r)
            prev_absorber = None
            pending = []
            for ch in range(NCH):
                po = ps_o.tile([Co + 1, 512], f32, tag="po")
                last_exp = None
                first_corr = None
                for tt in range(MT // 2):
                    pc = ps_c.tile([128, 1024], f32, tag="pc")
                    for k in range(2):
                        mm = nc.tensor.matmul(
                            pc[:, k * 512:(k + 1) * 512],
                            p_t[:, (2 * tt + k) * 128:(2 * tt + k + 1) * 128],
                            rng_t[:, ch * 512:(ch + 1) * 512],
                            start=True, stop=True,
                        )
                        if first_corr is None:
                            first_corr = mm
                    et = epool.tile([128, 1024], bf16)
                    second_last_exp = last_exp
                    last_exp = nc.scalar.activation(et, pc, Exp)
                    if prev_absorber is not None:
                        # order this chunk's exps after the previous chunk's
                        # absorber (same engine; no semaphore emitted)
                        tile.add_dep_helper(
                            last_exp.ins, prev_absorber.ins, sync=False,
                            reason="exp after absorber",
                        )
                    for k in range(2):
                        t = 2 * tt + k
                        nc.tensor.matmul(
                            po, vhat_t[:, t, :], et[:, k * 512:(k + 1) * 512],
                            start=(t == 0), stop=(t == MT - 1),
                        )
                # ACT self-tick absorber: a sync edge to the chunk's second-to-
                # last exp advances ACT's observed self-semaphore past (almost)
                # every exp of this chunk, so next-chunk et-slot WAW waits are
                # elided (the one uncovered exp costs a single EVSEM). Waiting
                # on the second-to-last exp hides the completion-semaphore
                # round-trip behind the last exp's execution.
                absorber = nc.scalar.copy(ab_dst[0:1, ch:ch + 1], ab_src[:])
                tile.add_dep_helper(
                    absorber.ins, second_last_exp.ins, sync=True,
                    reason="ACT self-tick absorber",
                )
                prev_absorber = absorber
                pending.append((ch, po))
                if len(pending) > 2:
                    postamble(*pending.pop(0), pe_pin=first_corr)
                if ch == NCH - 1:
                    # overlap all but the final postamble with this chunk
                    while len(pending) > 1:
                        postamble(*pending.pop(0), pe_pin=first_corr)
            postamble(*pending.pop(0), pe_pin=None)

    # Bacc defers register allocation etc. to compile(); finalize() runs it.
    nc.finalize()
    return nc


def _prepare(range_x, img, wq, bq, wk, bk, wv, bv, wc, bc,
             bn_gamma, bn_beta, bn_mean, bn_var):
    """Build (or fetch) the Bass program and the 8 per-core input maps."""
    import sys
    if "/opt/trn_rl_repo" not in sys.path:
        sys.path.insert(0, "/opt/trn_rl_repo")
    import ml_dtypes

    range_x = np.asarray(range_x, np.float32)
    img = np.asarray(img, np.float32)
    wq = np.asarray(wq, np.float32)
    bq = np.asarray(bq, np.float32)
    wk = np.asarray(wk, np.float32)
    bk = np.asarray(bk, np.float32)
    wv = np.asarray(wv, np.float32)
    bv = np.asarray(bv, np.float32)
    wc = np.asarray(wc, np.float32)
    bc = np.asarray(bc, np.float32)
    bn_gamma = np.asarray(bn_gamma, np.float32)
    bn_beta = np.asarray(bn_beta, np.float32)
    bn_mean = np.asarray(bn_mean, np.float32)
    bn_var = np.asarray(bn_var, np.float32)

    B, C, W, H = range_x.shape
    N = W * H
    NQ = N // 2
    Co = wq.shape[0]

    # Host-side weight folding (all tiny).
    inv = bn_gamma / np.sqrt(bn_var + BN_EPS)
    wcp = inv[:, None] * wc                                   # [C, Co]
    bcc = inv * bc + bn_beta - bn_mean * inv + wcp @ bv       # [C]
    at = wk.T @ wq                                            # lhsT for P-gen
    wvg = np.concatenate([wv.T, (wk.T @ bq)[:, None]], axis=1)  # [C, Co+1]
    wct = wcp.T                                               # [Co, C]

    key = (C, N, NQ, Co)
    if key not in _CACHE:
        _CACHE[key] = _build_program(C, N, NQ, Co)
    nc = _CACHE[key]

    n_cores = 8
    in_maps = []
    for core in range(n_cores):
        b, h = core // 2, core % 2
        im = img[b].reshape(C, N)
        x32 = np.zeros((C, _XIN32_W), np.float32)
        x32[:, _OFF32_IMQ:_OFF32_IMQ + NQ] = im[:, h * NQ:(h + 1) * NQ]
        x32[:, _OFF32_BCC] = bcc
        x16 = np.zeros((C, _XIN16_W), np.float16)
        x16[:, _OFF16_IMG:_OFF16_IMG + N] = im
        x16[:, _OFF16_RNG:_OFF16_RNG + NQ] = \
            range_x[b].reshape(C, N)[:, h * NQ:(h + 1) * NQ]
        x16[:, _OFF16_AT:_OFF16_AT + C] = at
        x16[:, _OFF16_WVG:_OFF16_WVG + Co + 1] = wvg
        xbf = np.zeros((C, _XINBF_W), ml_dtypes.bfloat16)
        xbf[0:Co, _OFFBF_WCT:_OFFBF_WCT + C] = wct.astype(ml_dtypes.bfloat16)
        in_maps.append({"x32": x32, "x16": x16, "xbf": xbf})

    return nc, in_maps, (B, C, W, H, N, NQ)


def kernel(range_x, img, wq, bq, wk, bk, wv, bv, wc, bc,
           bn_gamma, bn_beta, bn_mean, bn_var):
    import sys
    if "/opt/trn_rl_repo" not in sys.path:
        sys.path.insert(0, "/opt/trn_rl_repo")
    from concourse.bass_utils import run_bass_kernel_spmd

    nc, in_maps, (B, C, W, H, N, NQ) = _prepare(
        range_x, img, wq, bq, wk, bk, wv, bv, wc, bc,
        bn_gamma, bn_beta, bn_mean, bn_var)

    global _LAST_RESULTS
    _LAST_RESULTS = run_bass_kernel_spmd(nc, in_maps, list(range(8)))
    res = _LAST_RESULTS.results

    out = np.empty((B, C, N), np.float32)
    for core in range(8):
        b, h = core // 2, core % 2
        out[b, :, h * NQ:(h + 1) * NQ] = res[core]["out"]
    return out.reshape(B, C, W, H)


# revision 54
# speedup vs baseline: 1.2060x; 1.0153x over previous
"""Trainium2 Bass kernel for the nn_Attention problem (non-local attention block).

Reference computation (per batch b, with N = W*H spatial positions):
    q = wq @ r + bq                # [Co, N] from range_x
    k = wk @ i + bk                # [Co, N] from img
    corr[n, m] = q[:, n] . k[:, m]
    attn = softmax_m(corr)
    v = wv @ i + bv
    out = v @ attn^T               # [Co, N]
    y = relu(BN(wc @ out + bc))
    result = img + y

Algebraic restructuring used here:
    corr[n, m] = r_n^T A i_m + u[m] + w[n] + const,  A = wq^T wk
  - w[n] and const are per-query-row constants -> cancel in softmax. Dropped.
  - u[m] = (wk^T bq) . i_m varies per key -> kept, folded into Vhat as a
    per-key e^u scaling (exp(l+u) = exp(l) * e^u).
  - P = A @ i is precomputed on-device ([C, N]); logits tile = P_tile^T @ r
    contracts over C=128, fully using the 128-deep PE array (the naive
    q^T k contraction is only Co=64 deep).
  - softmax max-subtraction is skipped: logits ~ N(0, 64), max |logit| < ~60,
    exp stays comfortably inside fp32/bf16 range.
  - Normalization is deferred: Vhat = [v * e^u ; e^u] (65 rows). Then
    out_un = Vhat^T @ E where E = exp(P^T r); row 64 of out_un is the
    softmax denominator. v's bias bv is folded into the final projection
    bias on host.
  - BN (inference) + conv biases fold into wc' and bc'' on host.

Precision (validated numerically, end-to-end rel err ~5e-3):
  - logits path in fp16 (fp32 matmuls lower to TWO 2-cycle/column passes on
    the PE -> 4x the cost of a 16-bit matmul; fp16's 10-bit mantissa keeps
    the logit error ~8x below bf16)
  - attention-value path in bf16 (E spans e^-60..e^60: needs bf16 range)
  - softmax denominator/normalization in fp32

Sharding: 8 cores = 4 batches x 2 query-halves. Keys/values are the full
N=4096 per core; queries are a 2048-slice. No cross-core communication.

Engine discipline: walrus allows only ONE semaphore sync-wait per ISA
instruction (Bacc legalizes overflow into EventSemaphore preludes, but each
EVSEM costs an issue slot on the engine) so the kernel is structured so
nearly every instruction has at most one fresh semaphore dependency:
preamble PSUM->SBUF copies run on ACT, the postamble runs on DVE, every
PSUM tile gets one full-range reader, et tiles are never recycled within a
chunk, and a per-chunk ACT "absorber" advances ACT's observed self-tick so
cross-chunk et-slot WAW waits elide.
"""

import numpy as np

BN_EPS = 1e-5

_CACHE: dict = {}
_LAST_RESULTS = None  # BassKernelResults of the most recent run (for profiling)

# packed fp32 input layout (elements per partition row)
_OFF32_IMQ = 0       # [128, 2048] residual img (query half)
_OFF32_BCC = 2048    # [128, 1] folded output bias
_XIN32_W = 2049
# packed fp16 input layout
_OFF16_IMG = 0       # [128, 4096]
_OFF16_RNG = 4096    # [128, 2048]
_OFF16_AT = 6144     # [128, 128]
_OFF16_WVG = 6272    # [128, 65]
_XIN16_W = 6337
# packed bf16 input layout
_OFFBF_WCT = 0       # [64, 128]
_XINBF_W = 128


def _build_program(C: int, N: int, NQ: int, Co: int):
    import concourse.bass as bass
    import concourse.tile as tile
    from concourse import bacc, mybir

    f32 = mybir.dt.float32
    f16 = mybir.dt.float16
    bf16 = mybir.dt.bfloat16
    Exp = mybir.ActivationFunctionType.Exp
    Copy = mybir.ActivationFunctionType.Copy
    Add = mybir.AluOpType.add
    Max = mybir.AluOpType.max

    MT = N // 128      # key tiles (32)
    NCH = NQ // 512    # query chunks per core (4)
    PCH = N // 512     # chunks for P generation (8)

    # Bacc (not raw Bass): its compile() legalizes multi-wait instructions
    # into EventSemaphore preludes (TRN2 allows 1 sync wait per instruction).
    nc = bacc.Bacc()
    x32_d = nc.declare_dram_parameter("x32", [C, _XIN32_W], f32, isOutput=False)
    x16_d = nc.declare_dram_parameter("x16", [C, _XIN16_W], f16, isOutput=False)
    xbf_d = nc.declare_dram_parameter("xbf", [C, _XINBF_W], bf16, isOutput=False)
    out_d = nc.declare_dram_parameter("out", [C, NQ], f32, isOutput=True)

    with tile.TileContext(nc) as tc:
        with (
            tc.tile_pool(name="const", bufs=1) as cpool,
            tc.tile_pool(name="work", bufs=3) as wpool,
            # One et slot per exp within a chunk: an ACT->ACT WAW on a
            # recycled et slot would emit an ACT self-wait on the exp.
            tc.tile_pool(name="et", bufs=MT // 2) as epool,
            # PSUM: "pc" 2x2 banks (1024-wide logit tiles halve the ACT
            # per-instruction overhead), "pvy" 1, "po" 3 (postambles run two
            # chunks behind the main loop) -> 8 of 8.
            tc.tile_pool(name="ps_c", bufs=2, space="PSUM") as ps_c,
            tc.tile_pool(name="ps_vy", bufs=1, space="PSUM") as ps_vy,
            tc.tile_pool(name="ps_o", bufs=3, space="PSUM") as ps_o,
        ):
            # Chunked input DMAs: weights+queries first (small), then img in
            # quarters, so stage A starts ~2us in instead of after one big
            # 16us transfer. Separate DMA instructions land on separate HWDGE
            # queues and run in parallel.
            x16 = cpool.tile([C, _XIN16_W], f16)
            nc.sync.dma_start(x16[:, _OFF16_RNG:], x16_d[:, _OFF16_RNG:])
            for j in range(4):
                nc.sync.dma_start(
                    x16[:, j * 1024:(j + 1) * 1024],
                    x16_d[:, j * 1024:(j + 1) * 1024],
                )
            x32 = cpool.tile([C, _XIN32_W], f32)
            nc.sync.dma_start(x32[:], x32_d[:])
            xbf = cpool.tile([C, _XINBF_W], bf16)
            nc.sync.dma_start(xbf[:], xbf_d[:])
            imq_t = x32[:, _OFF32_IMQ:_OFF32_IMQ + NQ]
            bcc_t = x32[:, _OFF32_BCC:_OFF32_BCC + 1]
            img_t = x16[:, _OFF16_IMG:_OFF16_IMG + N]
            rng_t = x16[:, _OFF16_RNG:_OFF16_RNG + NQ]
            at_t = x16[:, _OFF16_AT:_OFF16_AT + C]
            wvg_t = x16[:, _OFF16_WVG:_OFF16_WVG + Co + 1]
            wct_t = xbf[0:Co, _OFFBF_WCT:_OFFBF_WCT + C]

            p_t = cpool.tile([C, N], f16)          # P = A @ img
            vhat_t = cpool.tile([128, MT, Co + 1], bf16)  # [v*eu ; eu] key-major
            eu_t = cpool.tile([128, MT], f32)      # e^u (fp32, for ACT scale)
            ab_src = cpool.tile([1, 1], f32)       # absorber scratch (see below)
            ab_dst = cpool.tile([1, NCH], f32)
            # single staging buffer + one output DMA (fewer HWDGE queue sems)
            fin_all = cpool.tile([C, NQ], f32)
            ones_t = cpool.tile([1, Co], f32)
            nc.vector.memset(ones_t[:], 1.0)
            bcc2_t = cpool.tile([C, 1], f32)
            nc.vector.tensor_copy(bcc2_t[:], bcc_t)
            nc.scalar.activation(ab_src[:], x32[0:1, 0:1], Copy, scale=0.0)



            # Stage A: P = A @ img  (lhsT = A^T, stationary across all chunks)
            # NOTE: each PSUM tile needs one reader covering the FULL written
            # range, else Tile keeps the PE writer in the slot release set and
            # the next matmul on that slot carries two sem waits.
            for j in range(PCH // 2):
                ps = ps_c.tile([128, 1024], f32, tag="pc")
                for k in range(2):
                    nc.tensor.matmul(
                        ps[:, k * 512:(k + 1) * 512], at_t,
                        img_t[:, (2 * j + k) * 512:(2 * j + k + 1) * 512],
                        start=True, stop=True,
                    )
                nc.vector.tensor_copy(p_t[:, j * 1024:(j + 1) * 1024], ps[:])

            # Stage B: per key-tile t: [v^T | u] = img_tile^T @ [wv^T | g],
            # then vhat = [v * e^u | e^u]. Seven tile outputs are packed per
            # PSUM bank (7 x 65 x 4B < 2KB) so the PSUM slot round-trip count
            # is 5 instead of 16 -- with per-tile round trips this stage
            # dribbled through the main loop and stalled the po matmuls.
            t0 = 0
            while t0 < MT:
                cnt = min(7, MT - t0)
                pvb = ps_vy.tile([128, 512], f32, tag="pvy")
                for i in range(cnt):
                    nc.tensor.matmul(
                        pvb[:, i * 65:(i + 1) * 65],
                        img_t[:, (t0 + i) * 128:(t0 + i + 1) * 128], wvg_t,
                        start=True, stop=True,
                    )
                pv3 = pvb[:, 0:cnt * 65].rearrange("p (t c) -> p t c", c=65)
                nc.vector.tensor_copy(
                    vhat_t[:, t0:t0 + cnt, :].rearrange("p t c -> p (t c)"),
                    pvb[:, 0:cnt * 65],
                )
                nc.scalar.activation(eu_t[:, t0:t0 + cnt], pv3[:, :, Co], Exp)
                nc.vector.tensor_copy(
                    vhat_t[:, t0:t0 + cnt, Co], eu_t[:, t0:t0 + cnt]
                )
                for i in range(cnt):
                    t = t0 + i
                    nc.vector.tensor_scalar_mul(
                        vhat_t[:, t, 0:Co], vhat_t[:, t, 0:Co], eu_t[:, t:t + 1]
                    )
                t0 += cnt

            def postamble(ch, po, pe_pin):
                # Normalize, project, BN+relu, residual (DVE + PE). Runs
                # software-pipelined two chunks behind the main loop; the two
                # PE matmuls are additionally pinned (no-sync ordering edges)
                # after `pe_pin` so they never stall PE's stream behind the
                # DVE reciprocal chain (a >3.4us PE gap re-throttles the HAM
                # clock gate).
                recip = wpool.tile([1, 512], f32, tag="recip")
                nc.vector.reciprocal(recip[:], po[Co:Co + 1, :])
                # Broadcast recip across partitions via a K=1 PE outer product.
                po2 = ps_vy.tile([Co, 512], f32, tag="pvy")
                mm1 = nc.tensor.matmul(po2, ones_t[:], recip[:],
                                       start=True, stop=True)
                rb = wpool.tile([Co, 512], f32, tag="rb")
                nc.vector.tensor_copy(rb[:], po2[:])
                onorm = wpool.tile([Co, 512], bf16, tag="onorm")
                nc.vector.tensor_mul(onorm[:], po[0:Co, :], rb[:])
                py = ps_vy.tile([128, 512], f32, tag="pvy")
                mm2 = nc.tensor.matmul(py, wct_t, onorm[:], start=True, stop=True)
                if pe_pin is not None:
                    for mm in (mm1, mm2):
                        tile.add_dep_helper(
                            mm.ins, pe_pin.ins, sync=False,
                            reason="postamble PE after next-next chunk start",
                        )
                yr = wpool.tile([128, 512], f32, tag="yr")
                nc.vector.tensor_scalar(yr[:], py[:], bcc2_t[:], 0.0, Add, Max)
                nc.vector.tensor_add(
                    fin_all[:, ch * 512:(ch + 1) * 512], yr[:],
                    imq_t[:, ch * 512:(ch + 1) * 512],
                )
                nc.sync.dma_start(
                    out_d[:, ch * 512:(ch + 1) * 512],
                    fin_all[:, ch * 512:(ch + 1) * 512],
                )

            # Main loop: per query chunk, accumulate Vhat^T @ exp(P^T r)
            prev_absorber = None
            pending = []
            for ch in range(NCH):
                po = ps_o.tile([Co + 1, 512], f32, tag="po")
                last_exp = None
                first_corr = None
                for tt in range(MT // 2):
                    pc = ps_c.tile([128, 1024], f32, tag="pc")
                    for k in range(2):
                        mm = nc.tensor.matmul(
                            pc[:, k * 512:(k + 1) * 512],
                            p_t[:, (2 * tt + k) * 128:(2 * tt + k + 1) * 128],
                            rng_t[:, ch * 512:(ch + 1) * 512],
                            start=True, stop=True,
                        )
                        if first_corr is None:
                            first_corr = mm
                    et = epool.tile([128, 1024], bf16)
                    second_last_exp = last_exp
                    last_exp = nc.scalar.activation(et, pc, Exp)
                    if prev_absorber is not None:
                        # order this chunk's exps after the previous chunk's
                        # absorber (same engine; no semaphore emitted)
                        tile.add_dep_helper(
                            last_exp.ins, prev_absorber.ins, sync=False,
                            reason="exp after absorber",
                        )
                    for k in range(2):
                        t = 2 * tt + k
                        nc.tensor.matmul(
                            po, vhat_t[:, t, :], et[:, k * 512:(k + 1) * 512],
                            start=(t == 0), stop=(t == MT - 1),
                        )
                # ACT self-tick absorber: a sync edge to the chunk's last exp
                # advances ACT's observed self-semaphore past every exp of
                # this chunk, so next-chunk et-slot WAW waits are elided
                # instead of landing as a second wait on an exp.
                absorber = nc.scalar.copy(ab_dst[0:1, ch:ch + 1], ab_src[:])
                tile.add_dep_helper(
                    absorber.ins, last_exp.ins, sync=True,
                    reason="ACT self-tick absorber",
                )
                prev_absorber = absorber
                pending.append((ch, po))
                if len(pending) > 2:
                    postamble(*pending.pop(0), pe_pin=first_corr)
                if ch == NCH - 1:
                    # overlap all but the final postamble with this chunk
                    while len(pending) > 1:
                        postamble(*pending.pop(0), pe_pin=first_corr)
            postamble(*pending.pop(0), pe_pin=None)

    # Bacc defers register allocation etc. to compile(); finalize() runs it.
    nc.finalize()
    return nc


def _prepare(range_x, img, wq, bq, wk, bk, wv, bv, wc, bc,
             bn_gamma, bn_beta, bn_mean, bn_var):
    """Build (or fetch) the Bass program and the 8 per-core input maps."""
    import sys
    if "/opt/trn_rl_repo" not in sys.path:
        sys.path.insert(0, "/opt/trn_rl_repo")
    import ml_dtypes

    range_x = np.asarray(range_x, np.float32)
    img = np.asarray(img, np.float32)
    wq = np.asarray(wq, np.float32)
    bq = np.asarray(bq, np.float32)
    wk = np.asarray(wk, np.float32)
    bk = np.asarray(bk, np.float32)
    wv = np.asarray(wv, np.float32)
    bv = np.asarray(bv, np.float32)
    wc = np.asarray(wc, np.float32)
    bc = np.asarray(bc, np.float32)
    bn_gamma = np.asarray(bn_gamma, np.float32)
    bn_beta = np.asarray(bn_beta, np.float32)
    bn_mean = np.asarray(bn_mean, np.float32)
    bn_var = np.asarray(bn_var, np.float32)

    B, C, W, H = range_x.shape
    N = W * H
    NQ = N // 2
    Co = wq.shape[0]

    # Host-side weight folding (all tiny).
    inv = bn_gamma / np.sqrt(bn_var + BN_EPS)
    wcp = inv[:, None] * wc                                   # [C, Co]
    bcc = inv * bc + bn_beta - bn_mean * inv + wcp @ bv       # [C]
    at = wk.T @ wq                                            # lhsT for P-gen
    wvg = np.concatenate([wv.T, (wk.T @ bq)[:, None]], axis=1)  # [C, Co+1]
    wct = wcp.T                                               # [Co, C]

    key = (C, N, NQ, Co)
    if key not in _CACHE:
        _CACHE[key] = _build_program(C, N, NQ, Co)
    nc = _CACHE[key]

    n_cores = 8
    in_maps = []
    for core in range(n_cores):
        b, h = core // 2, core % 2
        im = img[b].reshape(C, N)
        x32 = np.zeros((C, _XIN32_W), np.float32)
        x32[:, _OFF32_IMQ:_OFF32_IMQ + NQ] = im[:, h * NQ:(h + 1) * NQ]
        x32[:, _OFF32_BCC] = bcc
        x16 = np.zeros((C, _XIN16_W), np.float16)
        x16[:, _OFF16_IMG:_OFF16_IMG + N] = im
        x16[:, _OFF16_RNG:_OFF16_RNG + NQ] = \
            range_x[b].reshape(C, N)[:, h * NQ:(h + 1) * NQ]
        x16[:, _OFF16_AT:_OFF16_AT + C] = at
        x16[:, _OFF16_WVG:_OFF16_WVG + Co + 1] = wvg
        xbf = np.zeros((C, _XINBF_W), ml_dtypes.bfloat16)
        xbf[0:Co, _OFFBF_WCT:_OFFBF_WCT + C] = wct.astype(ml_dtypes.bfloat16)
        in_maps.append({"x32": x32, "x16": x16, "xbf": xbf})

    return nc, in_maps, (B, C, W, H, N, NQ)


def kernel(range_x, img, wq, bq, wk, bk, wv, bv, wc, bc,
           bn_gamma, bn_beta, bn_mean, bn_var):
    import sys
    if "/opt/trn_rl_repo" not in sys.path:
        sys.path.insert(0, "/opt/trn_rl_repo")
    from concourse.bass_utils import run_bass_kernel_spmd

    nc, in_maps, (B, C, W, H, N, NQ) = _prepare(
        range_x, img, wq, bq, wk, bk, wv, bv, wc, bc,
        bn_gamma, bn_beta, bn_mean, bn_var)

    global _LAST_RESULTS
    _LAST_RESULTS = run_bass_kernel_spmd(nc, in_maps, list(range(8)))
    res = _LAST_RESULTS.results

    out = np.empty((B, C, N), np.float32)
    for core in range(8):
        b, h = core // 2, core % 2
        out[b, :, h * NQ:(h + 1) * NQ] = res[core]["out"]
    return out.reshape(B, C, W, H)


# revision 57
# speedup vs baseline: 1.2376x; 1.0262x over previous
"""Trainium2 Bass kernel for the nn_Attention problem (non-local attention block).

Reference computation (per batch b, with N = W*H spatial positions):
    q = wq @ r + bq                # [Co, N] from range_x
    k = wk @ i + bk                # [Co, N] from img
    corr[n, m] = q[:, n] . k[:, m]
    attn = softmax_m(corr)
    v = wv @ i + bv
    out = v @ attn^T               # [Co, N]
    y = relu(BN(wc @ out + bc))
    result = img + y

Algebraic restructuring used here:
    corr[n, m] = r_n^T A i_m + u[m] + w[n] + const,  A = wq^T wk
  - w[n] and const are per-query-row constants -> cancel in softmax. Dropped.
  - u[m] = (wk^T bq) . i_m varies per key -> kept, folded into Vhat as a
    per-key e^u scaling (exp(l+u) = exp(l) * e^u).
  - P = A @ i is precomputed on-device ([C, N]); logits tile = P_tile^T @ r
    contracts over C=128, fully using the 128-deep PE array (the naive
    q^T k contraction is only Co=64 deep).
  - softmax max-subtraction is skipped: logits ~ N(0, 64), max |logit| < ~60,
    exp stays comfortably inside fp32/bf16 range.
  - Normalization is deferred: Vhat = [v * e^u ; e^u] (65 rows). Then
    out_un = Vhat^T @ E where E = exp(P^T r); row 64 of out_un is the
    softmax denominator. v's bias bv is folded into the final projection
    bias on host.
  - BN (inference) + conv biases fold into wc' and bc'' on host.

Precision (validated numerically, end-to-end rel err ~5e-3):
  - logits path in fp16 (fp32 matmuls lower to TWO 2-cycle/column passes on
    the PE -> 4x the cost of a 16-bit matmul; fp16's 10-bit mantissa keeps
    the logit error ~8x below bf16)
  - attention-value path in bf16 (E spans e^-60..e^60: needs bf16 range)
  - softmax denominator/normalization in fp32

Sharding: 8 cores = 4 batches x 2 query-halves. Keys/values are the full
N=4096 per core; queries are a 2048-slice. No cross-core communication.

Engine discipline: walrus allows only ONE semaphore sync-wait per ISA
instruction (Bacc legalizes overflow into EventSemaphore preludes, but each
EVSEM costs an issue slot on the engine) so the kernel is structured so
nearly every instruction has at most one fresh semaphore dependency:
preamble PSUM->SBUF copies run on ACT, the postamble runs on DVE, every
PSUM tile gets one full-range reader, et tiles are never recycled within a
chunk, and a per-chunk ACT "absorber" advances ACT's observed self-tick so
cross-chunk et-slot WAW waits elide.
"""

import numpy as np

BN_EPS = 1e-5

_CACHE: dict = {}
_LAST_RESULTS = None  # BassKernelResults of the most recent run (for profiling)

# packed fp32 input layout (elements per partition row)
_OFF32_IMQ = 0       # [128, 2048] residual img (query half)
_OFF32_BCC = 2048    # [128, 1] folded output bias
_XIN32_W = 2049
# packed fp16 input layout
_OFF16_IMG = 0       # [128, 4096]
_OFF16_RNG = 4096    # [128, 2048]
_OFF16_AT = 6144     # [128, 128]
_OFF16_WVG = 6272    # [128, 65]
_XIN16_W = 6337
# packed bf16 input layout
_OFFBF_WCT = 0       # [64, 128]
_XINBF_W = 128


def _build_program(C: int, N: int, NQ: int, Co: int):
    import concourse.bass as bass
    import concourse.tile as tile
    from concourse import bacc, mybir

    f32 = mybir.dt.float32
    f16 = mybir.dt.float16
    bf16 = mybir.dt.bfloat16
    Exp = mybir.ActivationFunctionType.Exp
    Copy = mybir.ActivationFunctionType.Copy
    Add = mybir.AluOpType.add
    Max = mybir.AluOpType.max
    Div = mybir.AluOpType.divide

    MT = N // 128      # key tiles (32)
    NCH = NQ // 512    # query chunks per core (4)
    PCH = N // 512     # chunks for P generation (8)

    # Bacc (not raw Bass): its compile() legalizes multi-wait instructions
    # into EventSemaphore preludes (TRN2 allows 1 sync wait per instruction).
    nc = bacc.Bacc()
    x32_d = nc.declare_dram_parameter("x32", [C, _XIN32_W], f32, isOutput=False)
    x16_d = nc.declare_dram_parameter("x16", [C, _XIN16_W], f16, isOutput=False)
    xbf_d = nc.declare_dram_parameter("xbf", [C, _XINBF_W], bf16, isOutput=False)
    out_d = nc.declare_dram_parameter("out", [C, NQ], f32, isOutput=True)

    with tile.TileContext(nc) as tc:
        with (
            tc.tile_pool(name="const", bufs=1) as cpool,
            tc.tile_pool(name="work", bufs=3) as wpool,
            # One et slot per exp within a chunk: an ACT->ACT WAW on a
            # recycled et slot would emit an ACT self-wait on the exp.
            tc.tile_pool(name="et", bufs=MT // 2) as epool,
            # PSUM: "pc" 2x2 banks (1024-wide logit tiles halve the ACT
            # per-instruction overhead), "pvy" 1, "po" 3 (postambles run two
            # chunks behind the main loop) -> 8 of 8.
            tc.tile_pool(name="ps_c", bufs=2, space="PSUM") as ps_c,
            tc.tile_pool(name="ps_vy", bufs=1, space="PSUM") as ps_vy,
            tc.tile_pool(name="ps_o", bufs=3, space="PSUM") as ps_o,
        ):
            # Chunked input DMAs: weights+queries first (small), then img in
            # quarters, so stage A starts ~2us in instead of after one big
            # 16us transfer. Separate DMA instructions land on separate HWDGE
            # queues and run in parallel.
            x16 = cpool.tile([C, _XIN16_W], f16)
            nc.sync.dma_start(x16[:, _OFF16_RNG:], x16_d[:, _OFF16_RNG:])
            for j in range(4):
                nc.sync.dma_start(
                    x16[:, j * 1024:(j + 1) * 1024],
                    x16_d[:, j * 1024:(j + 1) * 1024],
                )
            x32 = cpool.tile([C, _XIN32_W], f32)
            nc.sync.dma_start(x32[:], x32_d[:])
            xbf = cpool.tile([C, _XINBF_W], bf16)
            nc.sync.dma_start(xbf[:], xbf_d[:])
            imq_t = x32[:, _OFF32_IMQ:_OFF32_IMQ + NQ]
            bcc_t = x32[:, _OFF32_BCC:_OFF32_BCC + 1]
            img_t = x16[:, _OFF16_IMG:_OFF16_IMG + N]
            rng_t = x16[:, _OFF16_RNG:_OFF16_RNG + NQ]
            at_t = x16[:, _OFF16_AT:_OFF16_AT + C]
            wvg_t = x16[:, _OFF16_WVG:_OFF16_WVG + Co + 1]
            wct_t = xbf[0:Co, _OFFBF_WCT:_OFFBF_WCT + C]

            p_t = cpool.tile([C, N], f16)          # P = A @ img
            vhat_t = cpool.tile([128, MT, Co + 1], bf16)  # [v*eu ; eu] key-major
            eu_t = cpool.tile([128, MT], f32)      # e^u (fp32, for ACT scale)
            ab_src = cpool.tile([1, 1], f32)       # absorber scratch (see below)
            ab_dst = cpool.tile([1, NCH], f32)
            # single staging buffer + one output DMA (fewer HWDGE queue sems)
            fin_all = cpool.tile([C, NQ], f32)
            ones_t = cpool.tile([1, Co], f32)
            nc.vector.memset(ones_t[:], 1.0)
            bcc2_t = cpool.tile([C, 1], f32)
            nc.vector.tensor_copy(bcc2_t[:], bcc_t)
            nc.scalar.activation(ab_src[:], x32[0:1, 0:1], Copy, scale=0.0)



            # Stage A: P = A @ img  (lhsT = A^T, stationary across all chunks)
            # NOTE: each PSUM tile needs one reader covering the FULL written
            # range, else Tile keeps the PE writer in the slot release set and
            # the next matmul on that slot carries two sem waits.
            for j in range(PCH // 2):
                ps = ps_c.tile([128, 1024], f32, tag="pc")
                for k in range(2):
                    nc.tensor.matmul(
                        ps[:, k * 512:(k + 1) * 512], at_t,
                        img_t[:, (2 * j + k) * 512:(2 * j + k + 1) * 512],
                        start=True, stop=True,
                    )
                nc.vector.tensor_copy(p_t[:, j * 1024:(j + 1) * 1024], ps[:])

            # Stage B: per key-tile t: [v^T | u] = img_tile^T @ [wv^T | g],
            # then vhat = [v * e^u | e^u]. Seven tile outputs are packed per
            # PSUM bank (7 x 65 x 4B < 2KB) so the PSUM slot round-trip count
            # is 5 instead of 16 -- with per-tile round trips this stage
            # dribbled through the main loop and stalled the po matmuls.
            t0 = 0
            while t0 < MT:
                cnt = min(7, MT - t0)
                pvb = ps_vy.tile([128, 512], f32, tag="pvy")
                for i in range(cnt):
                    nc.tensor.matmul(
                        pvb[:, i * 65:(i + 1) * 65],
                        img_t[:, (t0 + i) * 128:(t0 + i + 1) * 128], wvg_t,
                        start=True, stop=True,
                    )
                pv3 = pvb[:, 0:cnt * 65].rearrange("p (t c) -> p t c", c=65)
                nc.vector.tensor_copy(
                    vhat_t[:, t0:t0 + cnt, :].rearrange("p t c -> p (t c)"),
                    pvb[:, 0:cnt * 65],
                )
                nc.scalar.activation(eu_t[:, t0:t0 + cnt], pv3[:, :, Co], Exp)
                nc.vector.tensor_copy(
                    vhat_t[:, t0:t0 + cnt, Co], eu_t[:, t0:t0 + cnt]
                )
                for i in range(cnt):
                    t = t0 + i
                    nc.vector.tensor_scalar_mul(
                        vhat_t[:, t, 0:Co], vhat_t[:, t, 0:Co], eu_t[:, t:t + 1]
                    )
                t0 += cnt

            def postamble(ch, po, pe_pin):
                # Normalize, project, BN+relu, residual (DVE + PE). Runs
                # software-pipelined two chunks behind the main loop; the two
                # PE matmuls are additionally pinned (no-sync ordering edges)
                # after `pe_pin` so they never stall PE's stream behind the
                # DVE reciprocal chain (a >3.4us PE gap re-throttles the HAM
                # clock gate).
                recip = wpool.tile([1, 512], f32, tag="recip")
                nc.vector.reciprocal(recip[:], po[Co:Co + 1, :])
                # Broadcast recip across partitions via a K=1 PE outer product.
                po2 = ps_vy.tile([Co, 512], f32, tag="pvy")
                mm1 = nc.tensor.matmul(po2, ones_t[:], recip[:],
                                       start=True, stop=True)
                rb = wpool.tile([Co, 512], f32, tag="rb")
                nc.vector.tensor_copy(rb[:], po2[:])
                onorm = wpool.tile([Co, 512], bf16, tag="onorm")
                nc.vector.tensor_mul(onorm[:], po[0:Co, :], rb[:])
                py = ps_vy.tile([128, 512], f32, tag="pvy")
                mm2 = nc.tensor.matmul(py, wct_t, onorm[:], start=True, stop=True)
                if pe_pin is not None:
                    for mm in (mm1, mm2):
                        tile.add_dep_helper(
                            mm.ins, pe_pin.ins, sync=False,
                            reason="postamble PE after next-next chunk start",
                        )
                yr = wpool.tile([128, 512], f32, tag="yr")
                nc.vector.tensor_scalar(yr[:], py[:], bcc2_t[:], 0.0, Add, Max)
                nc.vector.tensor_add(
                    fin_all[:, ch * 512:(ch + 1) * 512], yr[:],
                    imq_t[:, ch * 512:(ch + 1) * 512],
                )
                nc.sync.dma_start(
                    out_d[:, ch * 512:(ch + 1) * 512],
                    fin_all[:, ch * 512:(ch + 1) * 512],
                )

            # Main loop: per query chunk, accumulate Vhat^T @ exp(P^T r)
            prev_absorber = None
            pending = []
            for ch in range(NCH):
                po = ps_o.tile([Co + 1, 512], f32, tag="po")
                last_exp = None
                first_corr = None
                for tt in range(MT // 2):
                    pc = ps_c.tile([128, 1024], f32, tag="pc")
                    for k in range(2):
                        mm = nc.tensor.matmul(
                            pc[:, k * 512:(k + 1) * 512],
                            p_t[:, (2 * tt + k) * 128:(2 * tt + k + 1) * 128],
                            rng_t[:, ch * 512:(ch + 1) * 512],
                            start=True, stop=True,
                        )
                        if first_corr is None:
                            first_corr = mm
                    et = epool.tile([128, 1024], bf16)
                    second_last_exp = last_exp
                    last_exp = nc.scalar.activation(et, pc, Exp)
                    if prev_absorber is not None:
                        # order this chunk's exps after the previous chunk's
                        # absorber (same engine; no semaphore emitted)
                        tile.add_dep_helper(
                            last_exp.ins, prev_absorber.ins, sync=False,
                            reason="exp after absorber",
                        )
                    for k in range(2):
                        t = 2 * tt + k
                        nc.tensor.matmul(
                            po, vhat_t[:, t, :], et[:, k * 512:(k + 1) * 512],
                            start=(t == 0), stop=(t == MT - 1),
                        )
                # ACT self-tick absorber: a sync edge to the chunk's last exp
                # advances ACT's observed self-semaphore past every exp of
                # this chunk, so next-chunk et-slot WAW waits are elided
                # instead of landing as a second wait on an exp.
                absorber = nc.scalar.copy(ab_dst[0:1, ch:ch + 1], ab_src[:])
                tile.add_dep_helper(
                    absorber.ins, last_exp.ins, sync=True,
                    reason="ACT self-tick absorber",
                )
                prev_absorber = absorber
                pending.append((ch, po))
                if len(pending) > 2:
                    postamble(*pending.pop(0), pe_pin=first_corr)
                if ch == NCH - 1:
                    # overlap all but the final postamble with this chunk
                    while len(pending) > 1:
                        postamble(*pending.pop(0), pe_pin=first_corr)
            postamble(*pending.pop(0), pe_pin=None)

    # Bacc defers register allocation etc. to compile(); finalize() runs it.
    nc.finalize()
    return nc


def _prepare(range_x, img, wq, bq, wk, bk, wv, bv, wc, bc,
             bn_gamma, bn_beta, bn_mean, bn_var):
    """Build (or fetch) the Bass program and the 8 per-core input maps."""
    import sys
    if "/opt/trn_rl_repo" not in sys.path:
        sys.path.insert(0, "/opt/trn_rl_repo")
    import ml_dtypes

    range_x = np.asarray(range_x, np.float32)
    img = np.asarray(img, np.float32)
    wq = np.asarray(wq, np.float32)
    bq = np.asarray(bq, np.float32)
    wk = np.asarray(wk, np.float32)
    bk = np.asarray(bk, np.float32)
    wv = np.asarray(wv, np.float32)
    bv = np.asarray(bv, np.float32)
    wc = np.asarray(wc, np.float32)
    bc = np.asarray(bc, np.float32)
    bn_gamma = np.asarray(bn_gamma, np.float32)
    bn_beta = np.asarray(bn_beta, np.float32)
    bn_mean = np.asarray(bn_mean, np.float32)
    bn_var = np.asarray(bn_var, np.float32)

    B, C, W, H = range_x.shape
    N = W * H
    NQ = N // 2
    Co = wq.shape[0]

    # Host-side weight folding (all tiny).
    inv = bn_gamma / np.sqrt(bn_var + BN_EPS)
    wcp = inv[:, None] * wc                                   # [C, Co]
    bcc = inv * bc + bn_beta - bn_mean * inv + wcp @ bv       # [C]
    at = wk.T @ wq                                            # lhsT for P-gen
    wvg = np.concatenate([wv.T, (wk.T @ bq)[:, None]], axis=1)  # [C, Co+1]
    wct = wcp.T                                               # [Co, C]

    key = (C, N, NQ, Co)
    if key not in _CACHE:
        _CACHE[key] = _build_program(C, N, NQ, Co)
    nc = _CACHE[key]

    n_cores = 8
    in_maps = []
    for core in range(n_cores):
        b, h = core // 2, core % 2
        im = img[b].reshape(C, N)
        x32 = np.zeros((C, _XIN32_W), np.float32)
        x32[:, _OFF32_IMQ:_OFF32_IMQ + NQ] = im[:, h * NQ:(h + 1) * NQ]
        x32[:, _OFF32_BCC] = bcc
        x16 = np.zeros((C, _XIN16_W), np.float16)
        x16[:, _OFF16_IMG:_OFF16_IMG + N] = im
        x16[:, _OFF16_RNG:_OFF16_RNG + NQ] = \
            range_x[b].reshape(C, N)[:, h * NQ:(h + 1) * NQ]
        x16[:, _OFF16_AT:_OFF16_AT + C] = at
        x16[:, _OFF16_WVG:_OFF16_WVG + Co + 1] = wvg
        xbf = np.zeros((C, _XINBF_W), ml_dtypes.bfloat16)
        xbf[0:Co, _OFFBF_WCT:_OFFBF_WCT + C] = wct.astype(ml_dtypes.bfloat16)
        in_maps.append({"x32": x32, "x16": x16, "xbf": xbf})

    return nc, in_maps, (B, C, W, H, N, NQ)


def kernel(range_x, img, wq, bq, wk, bk, wv, bv, wc, bc,
           bn_gamma, bn_beta, bn_mean, bn_var):
    import sys
    if "/opt/trn_rl_repo" not in sys.path:
        sys.path.insert(0, "/opt/trn_rl_repo")
    from concourse.bass_utils import run_bass_kernel_spmd

    nc, in_maps, (B, C, W, H, N, NQ) = _prepare(
        range_x, img, wq, bq, wk, bk, wv, bv, wc, bc,
        bn_gamma, bn_beta, bn_mean, bn_var)

    global _LAST_RESULTS
    _LAST_RESULTS = run_bass_kernel_spmd(nc, in_maps, list(range(8)))
    res = _LAST_RESULTS.results

    out = np.empty((B, C, N), np.float32)
    for core in range(8):
        b, h = core // 2, core % 2
        out[b, :, h * NQ:(h + 1) * NQ] = res[core]["out"]
    return out.reshape(B, C, W, H)
